# revision 12
# baseline (speedup 1.0000x reference)
"""DeepSeek sparse attention on 8 Trainium2 NeuronCores (Bass/Tile).

Two SPMD launches (down from three):

  A   (column/indexer-head-parallel): core c computes the 256-col slices
      of the q/k/v projections (emitted transposed, bf16/f16) AND its
      indexer head's relevance scores rel_c using HOST-FUSED indexer
      weights (Wq@Wq_ind, Wk@Wk_ind).  The fusion decouples the indexer
      from q_lin/k_lin, killing the baseline's launch 2 (which reloaded
      32MB/core of qT/kT).  All matmul inputs fp16 (same PE rate as
      f32r, half the DMA of f32, and 8x finer mantissa than bf16 --
      bf16-level noise flips borderline top-k keys, each flip costing
      ~1e-2 rel err).  PE order: indexer projections first, then indexer
      score groups interleaved into the q/k/v matmul stream so ACT relu
      latency never stalls PE.
  host: rel = sum_c w_c*rel_c; top-1024 -> selected mask; v transposed
      to key-major f16 and premultiplied by the mask; hi threshold vec.
  B   (attention-head-parallel): core c owns heads 2c, 2c+1: softmax
      attention with causal/local/selected masking + output-projection
      partial (f16).  PE issue order software-pipelined: scores of key
      tile kc+1 are issued before AV of kc, hiding exp/mask latency.
  host: out = sum_c partial_c.
"""

import math

import numpy as np

import concourse.bass as bass
import concourse.mybir as mybir
from concourse import bacc
from concourse.tile import TileContext
from concourse.bass_utils import run_bass_kernel_spmd

# Problem constants (hardcoded per contract)
HIDDEN = 2048
NUM_HEADS = 16
HEAD_DIM = 128
NUM_IND_HEADS = 8
IND_DIM = HIDDEN // NUM_IND_HEADS  # 256
MAX_SELECTED = 1024
LOCAL_WINDOW = 512
N_CORES = 8
SEQ = 2048

F32 = mybir.dt.float32
F32R = mybir.dt.float32r
F16 = mybir.dt.float16
FP32 = np.float32

_TRACE = {"on": False, "exec_ns": []}


def build_la(S=SEQ, H=HIDDEN, CS=HIDDEN // N_CORES):
    """Per-core: qT/kT/vT (CS, S) slices + indexer-head rel (S)."""
    nc = bacc.Bacc("TRN2", target_bir_lowering=False, debug=False)
    HT, MC, NQ, QT, DC = H // 128, CS // 128, S // 512, S // 128, IND_DIM // 128
    hidT = nc.dram_tensor("hidT", [H, S], F16, kind="ExternalInput")
    wq = nc.dram_tensor("wq", [H, CS], F16, kind="ExternalInput")
    wk = nc.dram_tensor("wk", [H, CS], F16, kind="ExternalInput")
    wv = nc.dram_tensor("wv", [H, CS], F16, kind="ExternalInput")
    wqi = nc.dram_tensor("wqi", [H, CS], F16, kind="ExternalInput")
    wki = nc.dram_tensor("wki", [H, CS], F16, kind="ExternalInput")
    qT = nc.dram_tensor("qT", [CS, S], F16, kind="ExternalOutput")
    kT = nc.dram_tensor("kT", [CS, S], F16, kind="ExternalOutput")
    vT = nc.dram_tensor("vT", [CS, S], F16, kind="ExternalOutput")
    rel = nc.dram_tensor("rel", [S], F32, kind="ExternalOutput")

    AF = mybir.ActivationFunctionType
    OP = mybir.AluOpType

    with TileContext(nc) as tc:
        with (
            tc.tile_pool(name="hid", bufs=1) as hpool,
            tc.tile_pool(name="wt", bufs=1) as wpool,
            tc.tile_pool(name="proj", bufs=1) as ppool,
            tc.tile_pool(name="ev", bufs=2) as opool,
            tc.tile_pool(name="scr", bufs=2) as scpool,
            tc.tile_pool(name="rc", bufs=2) as rcpool,
            tc.tile_pool(name="rm", bufs=1) as rmpool,
            tc.tile_pool(name="ps", bufs=1, space="PSUM") as pspool,
        ):
            def load_w(wdram):
                wr = wpool.tile([128, HT * CS], F16, name=f"w_{wdram.name}")
                nc.sync.dma_start(
                    out=wr.rearrange("p (t c) -> p t c", t=HT),
                    in_=wdram.rearrange("(t p) c -> p t c", p=128),
                )
                return wr

            def load_strip(t):
                hs = hpool.tile([128, S], F16, name=f"hid{t}")
                nc.sync.dma_start(out=hs, in_=hidT[t * 128:(t + 1) * 128, :])
                return hs

            def load_w_split(wdram):
                # first 128-row strip as its own small DMA so the very first
                # matmul (needing only strip 0) isn't gated on the full 1MB
                wr = wpool.tile([128, HT * CS], F16, name=f"w_{wdram.name}")
                nc.sync.dma_start(out=wr[:, 0:CS], in_=wdram[0:128, :])
                nc.sync.dma_start(
                    out=wr[:, CS:].rearrange("p (t c) -> p t c", t=HT - 1),
                    in_=wdram[128:, :].rearrange("(t p) c -> p t c", p=128),
                )
                return wr

            # DMA order: first operands for the indexer projections, then the
            # rest of hidden, then q/k/v weights (needed ~55us in).
            wqi_t = load_w_split(wqi)
            hids = [load_strip(0)]
            wki_t = load_w(wki)
            hids += [load_strip(t) for t in range(1, HT)]
            wq_t, wk_t, wv_t = load_w(wq), load_w(wk), load_w(wv)

            qpi = [ppool.tile([128, S], F16, name=f"qpi{d}") for d in range(DC)]
            kpi = [ppool.tile([128, S], F16, name=f"kpi{d}") for d in range(DC)]

            # ---- phase 1: indexer projections (hidden @ fused weights) ----
            for wt, dst in ((wqi_t, qpi), (wki_t, kpi)):
                for mc in range(MC):
                    psums = [
                        pspool.tile([128, 512], F32, tag=f"p{qc}", name=f"p{qc}")
                        for qc in range(NQ)
                    ]
                    for t in range(HT):
                        lhsT = wt[:, t * CS + mc * 128: t * CS + mc * 128 + 128]
                        for qc in range(NQ):
                            nc.tensor.matmul(
                                psums[qc], lhsT,
                                hids[t][:, qc * 512:(qc + 1) * 512],
                                start=(t == 0), stop=(t == HT - 1),
                            )
                    for qc in range(NQ):
                        nc.vector.tensor_copy(
                            dst[mc][:, qc * 512:(qc + 1) * 512], psums[qc]
                        )

            # ---- phase 2: q/k/v projections interleaved with score groups ----
            relmat = rmpool.tile([128, QT], F32, name="relmat")

            def gen_qkv():
                for wt, odram, odt in (
                    (wq_t, qT, F16), (wk_t, kT, F16), (wv_t, vT, F16),
                ):
                    for mc in range(MC):
                        psums = [
                            pspool.tile([128, 512], F32, tag=f"p{qc}",
                                        name=f"pp{qc}")
                            for qc in range(NQ)
                        ]
                        for t in range(HT):
                            lhsT = wt[:, t * CS + mc * 128:
                                      t * CS + mc * 128 + 128]
                            for qc in range(NQ):
                                nc.tensor.matmul(
                                    psums[qc], lhsT,
                                    hids[t][:, qc * 512:(qc + 1) * 512],
                                    start=(t == 0), stop=(t == HT - 1),
                                )
                            if t == 7:
                                yield
                        for qc in range(NQ):
                            ot = opool.tile([128, 512], odt, tag=f"ot{qc}",
                                            name=f"ot{qc}")
                            nc.vector.tensor_copy(ot, psums[qc])
                            nc.sync.dma_start(
                                out=odram[mc * 128:(mc + 1) * 128,
                                          qc * 512:(qc + 1) * 512],
                                in_=ot,
                            )
                        yield

            qkv = gen_qkv()
            for qt in range(QT):
                # score group qt: rel rows for q-tile qt
                sps = [
                    pspool.tile([128, 512], F32, tag=f"s{kc}", name=f"s{kc}")
                    for kc in range(NQ)
                ]
                for kc in range(NQ):
                    for d in range(DC):
                        nc.tensor.matmul(
                            sps[kc],
                            qpi[d][:, qt * 128:(qt + 1) * 128],
                            kpi[d][:, kc * 512:(kc + 1) * 512],
                            start=(d == 0), stop=(d == DC - 1),
                        )
                relcols = rcpool.tile([128, NQ], F32, tag="relcols",
                                      name="relcols")
                for kc in range(NQ):
                    scratch = scpool.tile([128, 512], F16, tag="scratch",
                                          name="scratch")
                    nc.scalar.activation(
                        scratch, sps[kc], AF.Relu,
                        accum_out=relcols[:, kc:kc + 1],
                    )
                nc.vector.tensor_reduce(
                    relmat[:, qt:qt + 1], relcols, axis=mybir.AxisListType.X,
                    op=OP.add,
                )
                next(qkv, None)
            for _ in qkv:
                pass

            nc.sync.dma_start(
                out=rel.rearrange("(t p) -> p t", p=128), in_=relmat
            )
    nc.compile()
    return nc


def build_lb(S=SEQ, H=HIDDEN, NHC=NUM_HEADS // N_CORES, HD=HEAD_DIM,
             window=LOCAL_WINDOW):
    """Per-core (attention heads): partial (S, H) f16 = softmax-attn @ Wo rows."""
    nc = bacc.Bacc("TRN2", target_bir_lowering=False, debug=False)
    KC, NQ, QT, OCC = S // 128, S // 512, S // 128, H // 512
    qTh = nc.dram_tensor("qTh", [NHC * HD, S], F16, kind="ExternalInput")
    kTh = nc.dram_tensor("kTh", [NHC * HD, S], F16, kind="ExternalInput")
    vh = nc.dram_tensor("vh", [S, NHC * HD], F16, kind="ExternalInput")
    vslh = nc.dram_tensor("vslh", [S, NHC * HD], F16, kind="ExternalInput")
    woh = nc.dram_tensor("woh", [NHC * HD, H], F16, kind="ExternalInput")
    hivec = nc.dram_tensor("hivec", [S], F16, kind="ExternalInput")
    selv = nc.dram_tensor("selv", [S], F16, kind="ExternalInput")
    part = nc.dram_tensor("part", [S, H], F16, kind="ExternalOutput")

    scale = 1.0 / math.sqrt(HD)
    AF = mybir.ActivationFunctionType
    OP = mybir.AluOpType

    with TileContext(nc) as tc:
        with (
            tc.tile_pool(name="const", bufs=1) as cpool,
            tc.tile_pool(name="qk", bufs=1) as qkpool,
            tc.tile_pool(name="vv", bufs=1) as vpool,
            tc.tile_pool(name="et", bufs=2) as etpool,
            tc.tile_pool(name="aon", bufs=1) as aopool,
            tc.tile_pool(name="dr", bufs=2) as drpool,
            tc.tile_pool(name="ev", bufs=2) as evpool,
            tc.tile_pool(name="ps", bufs=1, space="PSUM") as pspool,
        ):
            # mask vectors first (needed by kc=0), then head 0 operands
            # (q/k then v/vsl, all needed by the first kc tiles), then head 1.
            hvec = cpool.tile([128, KC], F16, name="hvec")
            nc.sync.dma_start(out=hvec, in_=hivec.rearrange("(t p) -> p t", p=128))
            svec = cpool.tile([128, KC], F16, name="svec")
            nc.sync.dma_start(out=svec, in_=selv.rearrange("(t p) -> p t", p=128))
            qsb, ksb, vhf, vsl = [], [], [], []
            for h in range(NHC):
                q = qkpool.tile([128, S], F16, name=f"qsb{h}")
                nc.sync.dma_start(out=q, in_=qTh[h * HD:(h + 1) * HD, :])
                qsb.append(q)
                k = qkpool.tile([128, S], F16, name=f"ksb{h}")
                nc.sync.dma_start(out=k, in_=kTh[h * HD:(h + 1) * HD, :])
                ksb.append(k)
                vt = vpool.tile([128, KC * HD], F16, name=f"vhf{h}")
                nc.sync.dma_start(
                    out=vt.rearrange("p (t d) -> p t d", t=KC),
                    in_=vh[:, h * HD:(h + 1) * HD].rearrange(
                        "(t p) d -> p t d", p=128),
                )
                vhf.append(vt)
                vs = vpool.tile([128, KC * HD], F16, name=f"vsl{h}")
                nc.sync.dma_start(
                    out=vs.rearrange("p (t d) -> p t d", t=KC),
                    in_=vslh[:, h * HD:(h + 1) * HD].rearrange(
                        "(t p) d -> p t d", p=128),
                )
                vsl.append(vs)

            iota = cpool.tile([128, S], F16, name="iota")
            nc.gpsimd.iota(
                iota, pattern=[[1, S]], base=0, channel_multiplier=0,
                allow_small_or_imprecise_dtypes=True,
            )
            ones = cpool.tile([128, 1], F16, name="ones")
            nc.vector.memset(ones, 1.0)

            wsb = []
            for h in range(NHC):
                w = qkpool.tile([128, H], F16, name=f"wsb{h}")
                nc.sync.dma_start(out=w, in_=woh[h * HD:(h + 1) * HD, :])
                wsb.append(w)

            aon = [aopool.tile([128, S], F16, name=f"aon{h}")
                   for h in range(NHC)]

            def make_norm(h, avp, den128):
                def emit_norm(qc):
                    # den -> reciprocal -> partition-broadcast -> normalize;
                    # DVE/gpsimd only, so PE never waits on this chain except
                    # through the av/den bank reuse semaphores.
                    q0 = qc * 512
                    dq = drpool.tile([1, 512], F32, tag=f"dq{qc}",
                                     name=f"dq{qc}")
                    nc.vector.tensor_copy(dq, den128[32 * qc:32 * qc + 1, :])
                    rq = drpool.tile([1, 512], F32, tag=f"rq{qc}",
                                     name=f"rq{qc}")
                    rs = drpool.tile([1, 512], F32, tag=f"rs{qc}",
                                     name=f"rs{qc}")
                    nc.vector.reciprocal_approx_accurate(rq, dq, rs)
                    rbs = drpool.tile([128, 512], F32, tag="rbs", name="rbs")
                    nc.gpsimd.partition_broadcast(rbs, rq)
                    nc.vector.scalar_tensor_tensor(
                        aon[h][:, q0:q0 + 512], rbs, 1.0, avp[qc],
                        op0=OP.mult, op1=OP.mult,
                    )
                return emit_norm

            for h in range(NHC):
                avp = [
                    pspool.tile([128, 512], F32, tag=f"av{qc}", bufs=1,
                                name=f"av{qc}")
                    for qc in range(NQ)
                ]
                den128 = pspool.tile([128, 512], F32, tag="den", bufs=1,
                                     name="den128")
                emit_norm = make_norm(h, avp, den128)

                def emit_av_den(kc, qcs, far, ets):
                    for qc in qcs:
                        lhs_av = vsl[h] if far[qc] else vhf[h]
                        nc.tensor.matmul(
                            avp[qc], lhs_av[:, kc * 128:(kc + 1) * 128],
                            ets[qc], start=(kc == 0),
                            stop=(kc == (qc * 512 + 511) // 128),
                        )
                    for qc in qcs:
                        lhs_den = svec[:, kc:kc + 1] if far[qc] else ones
                        nc.tensor.matmul(
                            den128[32 * qc:32 * qc + 1, :], lhs_den, ets[qc],
                            start=(kc == 0),
                            stop=(kc == (qc * 512 + 511) // 128),
                            tile_position=(0, 32 * qc),
                        )
                    # a q-chunk whose last key tile just finished can be
                    # normalized now, overlapping the remaining kc loop
                    for qc in qcs:
                        if kc == (qc * 512 + 511) // 128:
                            emit_norm(qc)

                pend = None
                for kc in range(KC):
                    k0 = kc * 128
                    qcs = [qc for qc in range(NQ) if qc * 512 + 511 >= k0]
                    far = {qc: qc * 512 > k0 + 127 + window for qc in qcs}
                    ets = {}
                    for qc in qcs:
                        q0 = qc * 512
                        sps = pspool.tile([128, 512], F32, tag="sc", bufs=3,
                                          name="sps")
                        nc.tensor.matmul(
                            sps, ksb[h][:, k0:k0 + 128],
                            qsb[h][:, q0:q0 + 512], start=True, stop=True,
                        )
                        et = etpool.tile([128, 512], F16, tag=f"et{qc}",
                                         name=f"et{qc}")
                        ets[qc] = et
                        nc.scalar.activation(et, sps, AF.Exp, scale=scale)
                        if far[qc]:
                            continue  # sel-mask folded into vsl/svec operands
                        if q0 < k0 + 128:
                            # causal: zero where q < k
                            nc.gpsimd.affine_select(
                                out=et, in_=et, compare_op=OP.is_ge, fill=0.0,
                                base=q0 - k0, channel_multiplier=-1,
                                pattern=[[1, 512]],
                            )
                        if q0 + 511 > k0 + window:
                            nc.vector.scalar_tensor_tensor(
                                et, iota[:, q0:q0 + 512], hvec[:, kc:kc + 1],
                                et, op0=OP.is_le, op1=OP.mult,
                            )
                    if pend is not None:
                        emit_av_den(*pend)
                    pend = (kc, qcs, far, ets)
                emit_av_den(*pend)

            # output projection: partial = sum_h aon_h @ Wo rows.
            # oc=3 uses the den bank so no wops waits on the (late) av3
            # normalize read; qt order is free since all norms are done.
            wop_tags = ["av0", "av1", "av2", "den"]
            nev = 0
            for qt in range(QT):
                wops = [
                    pspool.tile([128, 512], F32, tag=wop_tags[oc], bufs=1,
                                name=f"wops{oc}")
                    for oc in range(OCC)
                ]
                for h in range(NHC):
                    for oc in range(OCC):
                        nc.tensor.matmul(
                            wops[oc], aon[h][:, qt * 128:(qt + 1) * 128],
                            wsb[h][:, oc * 512:(oc + 1) * 512],
                            start=(h == 0), stop=(h == NHC - 1),
                        )
                for oc in range(OCC):
                    ot = evpool.tile([128, 512], F16, tag=f"ot{oc}",
                                     name=f"ot{oc}")
                    nev += 1
                    if nev % 2 == 0:
                        nc.scalar.copy(ot, wops[oc])
                    else:
                        nc.vector.tensor_copy(ot, wops[oc])
                    nc.sync.dma_start(
                        out=part[qt * 128:(qt + 1) * 128,
                                 oc * 512:(oc + 1) * 512],
                        in_=ot,
                    )
    nc.compile()
    return nc


_CACHE = {}


def _get(name, builder, *args):
    key = (name,) + args
    if key not in _CACHE:
        _CACHE[key] = builder(*args)
    return _CACHE[key]


def _run(nc, in_maps):
    res = run_bass_kernel_spmd(
        nc, in_maps, core_ids=list(range(N_CORES)), trace=_TRACE["on"]
    )
    if _TRACE["on"] and res.exec_time_ns is not None:
        _TRACE["exec_ns"].append(res.exec_time_ns)
    return res.results


def kernel(hidden_states, Wq, Wk, Wv, Wo, Wq_ind, Wk_ind, head_weights,
           temperature_param):
    hidden_states = np.asarray(hidden_states, dtype=FP32)
    Wq, Wk, Wv, Wo = (np.asarray(a, dtype=FP32) for a in (Wq, Wk, Wv, Wo))
    Wq_ind = np.asarray(Wq_ind, dtype=FP32)
    Wk_ind = np.asarray(Wk_ind, dtype=FP32)
    head_weights = np.asarray(head_weights, dtype=FP32)

    B, S, H = hidden_states.shape
    assert B == 1 and H == HIDDEN and S == SEQ
    CS = H // N_CORES

    # fused indexer weights: qp = q_lin@Wq_ind = hidden@(Wq@Wq_ind)
    Wqi_f = Wq @ Wq_ind
    Wki_f = Wk @ Wk_ind

    hidT = np.ascontiguousarray(hidden_states[0].T).astype(np.float16)

    # ---- launch A: projections + indexer rel ----
    nca = _get("la", build_la, S, H, CS)
    ina = [
        {
            "hidT": hidT,
            "wq": np.ascontiguousarray(Wq[:, c * CS:(c + 1) * CS]).astype(np.float16),
            "wk": np.ascontiguousarray(Wk[:, c * CS:(c + 1) * CS]).astype(np.float16),
            "wv": np.ascontiguousarray(Wv[:, c * CS:(c + 1) * CS]).astype(np.float16),
            "wqi": np.ascontiguousarray(Wqi_f[:, c * CS:(c + 1) * CS]).astype(np.float16),
            "wki": np.ascontiguousarray(Wki_f[:, c * CS:(c + 1) * CS]).astype(np.float16),
        }
        for c in range(N_CORES)
    ]
    ra = _run(nca, ina)

    rel = np.zeros(S, dtype=np.float64)
    for c in range(N_CORES):
        rel += float(head_weights[c]) * np.asarray(ra[c]["rel"], dtype=np.float64)
    # exp(-temp) scaling is monotone; irrelevant for top-k selection.

    k_sel = min(MAX_SELECTED, S)
    top_idx = np.argpartition(-rel, k_sel - 1)[:k_sel]
    selected = np.zeros(S, dtype=bool)
    selected[top_idx] = True

    # ---- launch B: masked attention + output projection ----
    BIG = float(2 * S + 1024)
    hi = np.where(selected, BIG, np.arange(S, dtype=np.float64) + LOCAL_WINDOW)
    hi = hi.astype(np.float16)
    selv = selected.astype(np.float16)
    NHC = NUM_HEADS // N_CORES
    RW = NHC * HEAD_DIM

    ncb = _get("lb", build_lb, S, H, NHC, HEAD_DIM, LOCAL_WINDOW)
    inb = []
    for c in range(N_CORES):
        vhc = np.ascontiguousarray(
            np.asarray(ra[c]["vT"], dtype=np.float16).T)  # (S, 256) key-major
        inb.append({
            "qTh": np.asarray(ra[c]["qT"]),
            "kTh": np.asarray(ra[c]["kT"]),
            "vh": vhc,
            "vslh": np.ascontiguousarray(vhc * selv[:, None]),
            "woh": np.ascontiguousarray(Wo[c * RW:(c + 1) * RW]).astype(np.float16),
            "hivec": hi,
            "selv": selv,
        })
    rb = _run(ncb, inb)
    out = np.zeros((S, H), dtype=np.float32)
    for c in range(N_CORES):
        out += np.asarray(rb[c]["part"], dtype=np.float32)
    return out.reshape(B, S, H).astype(np.float32)


# revision 14
# speedup vs baseline: 1.0030x; 1.0030x over previous
"""DeepSeek sparse attention on 8 Trainium2 NeuronCores (Bass/Tile).

Two SPMD launches (down from three):

  A   (column/indexer-head-parallel): core c computes the 256-col slices
      of the q/k/v projections (emitted transposed, bf16/f16) AND its
      indexer head's relevance scores rel_c using HOST-FUSED indexer
      weights (Wq@Wq_ind, Wk@Wk_ind).  The fusion decouples the indexer
      from q_lin/k_lin, killing the baseline's launch 2 (which reloaded
      32MB/core of qT/kT).  All matmul inputs fp16 (same PE rate as
      f32r, half the DMA of f32, and 8x finer mantissa than bf16 --
      bf16-level noise flips borderline top-k keys, each flip costing
      ~1e-2 rel err).  PE order: indexer projections first, then indexer
      score groups interleaved into the q/k/v matmul stream so ACT relu
      latency never stalls PE.
  host: rel = sum_c w_c*rel_c; top-1024 -> selected mask; v transposed
      to key-major f16 and premultiplied by the mask; hi threshold vec.
  B   (attention-head-parallel): core c owns heads 2c, 2c+1: softmax
      attention with causal/local/selected masking + output-projection
      partial (f16).  PE issue order software-pipelined: scores of key
      tile kc+1 are issued before AV of kc, hiding exp/mask latency.
  host: out = sum_c partial_c.
"""

import math

import numpy as np

import concourse.bass as bass
import concourse.mybir as mybir
from concourse import bacc
from concourse.tile import TileContext
from concourse.bass_utils import run_bass_kernel_spmd

# Problem constants (hardcoded per contract)
HIDDEN = 2048
NUM_HEADS = 16
HEAD_DIM = 128
NUM_IND_HEADS = 8
IND_DIM = HIDDEN // NUM_IND_HEADS  # 256
MAX_SELECTED = 1024
LOCAL_WINDOW = 512
N_CORES = 8
SEQ = 2048

F32 = mybir.dt.float32
F32R = mybir.dt.float32r
F16 = mybir.dt.float16
FP32 = np.float32

_TRACE = {"on": False, "exec_ns": []}


def build_la(S=SEQ, H=HIDDEN, CS=HIDDEN // N_CORES):
    """Per-core: qT/kT/vT (CS, S) slices + indexer-head rel (S)."""
    nc = bacc.Bacc("TRN2", target_bir_lowering=False, debug=False)
    HT, MC, NQ, QT, DC = H // 128, CS // 128, S // 512, S // 128, IND_DIM // 128
    hidT = nc.dram_tensor("hidT", [H, S], F16, kind="ExternalInput")
    wq = nc.dram_tensor("wq", [H, CS], F16, kind="ExternalInput")
    wk = nc.dram_tensor("wk", [H, CS], F16, kind="ExternalInput")
    wv = nc.dram_tensor("wv", [H, CS], F16, kind="ExternalInput")
    wqi = nc.dram_tensor("wqi", [H, CS], F16, kind="ExternalInput")
    wki = nc.dram_tensor("wki", [H, CS], F16, kind="ExternalInput")
    qT = nc.dram_tensor("qT", [CS, S], F16, kind="ExternalOutput")
    kT = nc.dram_tensor("kT", [CS, S], F16, kind="ExternalOutput")
    vT = nc.dram_tensor("vT", [CS, S], F16, kind="ExternalOutput")
    rel = nc.dram_tensor("rel", [S], F32, kind="ExternalOutput")

    AF = mybir.ActivationFunctionType
    OP = mybir.AluOpType

    with TileContext(nc) as tc:
        with (
            tc.tile_pool(name="hid", bufs=1) as hpool,
            tc.tile_pool(name="wt", bufs=1) as wpool,
            tc.tile_pool(name="proj", bufs=1) as ppool,
            tc.tile_pool(name="ev", bufs=2) as opool,
            tc.tile_pool(name="scr", bufs=2) as scpool,
            tc.tile_pool(name="rc", bufs=2) as rcpool,
            tc.tile_pool(name="rm", bufs=1) as rmpool,
            tc.tile_pool(name="ps", bufs=1, space="PSUM") as pspool,
        ):
            def load_w(wdram):
                wr = wpool.tile([128, HT * CS], F16, name=f"w_{wdram.name}")
                nc.sync.dma_start(
                    out=wr.rearrange("p (t c) -> p t c", t=HT),
                    in_=wdram.rearrange("(t p) c -> p t c", p=128),
                )
                return wr

            def load_strip(t):
                hs = hpool.tile([128, S], F16, name=f"hid{t}")
                nc.sync.dma_start(out=hs, in_=hidT[t * 128:(t + 1) * 128, :])
                return hs

            # DMA order: the first matmul needs only wqi strip 0 (64KB) and
            # hid strip 0, so issue those two tiny/medium DMAs before the
            # bulk wqi load -- descriptor generation on the sync queue is
            # serial, and a big strided DMA ahead of strip 0 delays PE start.
            wqi_t = wpool.tile([128, HT * CS], F16, name="w_wqi")
            nc.sync.dma_start(out=wqi_t[:, 0:CS], in_=wqi[0:128, :])
            hids = [load_strip(0)]
            nc.sync.dma_start(
                out=wqi_t[:, CS:].rearrange("p (t c) -> p t c", t=HT - 1),
                in_=wqi[128:, :].rearrange("(t p) c -> p t c", p=128),
            )
            wki_t = load_w(wki)
            hids += [load_strip(t) for t in range(1, HT)]
            wq_t, wk_t, wv_t = load_w(wq), load_w(wk), load_w(wv)

            qpi = [ppool.tile([128, S], F16, name=f"qpi{d}") for d in range(DC)]
            kpi = [ppool.tile([128, S], F16, name=f"kpi{d}") for d in range(DC)]

            # ---- phase 1: indexer projections (hidden @ fused weights) ----
            for wt, dst in ((wqi_t, qpi), (wki_t, kpi)):
                for mc in range(MC):
                    psums = [
                        pspool.tile([128, 512], F32, tag=f"p{qc}", name=f"p{qc}")
                        for qc in range(NQ)
                    ]
                    for t in range(HT):
                        lhsT = wt[:, t * CS + mc * 128: t * CS + mc * 128 + 128]
                        for qc in range(NQ):
                            nc.tensor.matmul(
                                psums[qc], lhsT,
                                hids[t][:, qc * 512:(qc + 1) * 512],
                                start=(t == 0), stop=(t == HT - 1),
                            )
                    for qc in range(NQ):
                        nc.vector.tensor_copy(
                            dst[mc][:, qc * 512:(qc + 1) * 512], psums[qc]
                        )

            # ---- phase 2: q/k/v projections interleaved with score groups ----
            relmat = rmpool.tile([128, QT], F32, name="relmat")

            def gen_qkv():
                for wt, odram, odt in (
                    (wq_t, qT, F16), (wk_t, kT, F16), (wv_t, vT, F16),
                ):
                    for mc in range(MC):
                        psums = [
                            pspool.tile([128, 512], F32, tag=f"p{qc}",
                                        name=f"pp{qc}")
                            for qc in range(NQ)
                        ]
                        for t in range(HT):
                            lhsT = wt[:, t * CS + mc * 128:
                                      t * CS + mc * 128 + 128]
                            for qc in range(NQ):
                                nc.tensor.matmul(
                                    psums[qc], lhsT,
                                    hids[t][:, qc * 512:(qc + 1) * 512],
                                    start=(t == 0), stop=(t == HT - 1),
                                )
                            if t == 7:
                                yield
                        for qc in range(NQ):
                            ot = opool.tile([128, 512], odt, tag=f"ot{qc}",
                                            name=f"ot{qc}")
                            nc.vector.tensor_copy(ot, psums[qc])
                            nc.sync.dma_start(
                                out=odram[mc * 128:(mc + 1) * 128,
                                          qc * 512:(qc + 1) * 512],
                                in_=ot,
                            )
                        yield

            qkv = gen_qkv()
            for qt in range(QT):
                # score group qt: rel rows for q-tile qt
                sps = [
                    pspool.tile([128, 512], F32, tag=f"s{kc}", name=f"s{kc}")
                    for kc in range(NQ)
                ]
                for kc in range(NQ):
                    for d in range(DC):
                        nc.tensor.matmul(
                            sps[kc],
                            qpi[d][:, qt * 128:(qt + 1) * 128],
                            kpi[d][:, kc * 512:(kc + 1) * 512],
                            start=(d == 0), stop=(d == DC - 1),
                        )
                relcols = rcpool.tile([128, NQ], F32, tag="relcols",
                                      name="relcols")
                for kc in range(NQ):
                    scratch = scpool.tile([128, 512], F16, tag="scratch",
                                          name="scratch")
                    nc.scalar.activation(
                        scratch, sps[kc], AF.Relu,
                        accum_out=relcols[:, kc:kc + 1],
                    )
                nc.vector.tensor_reduce(
                    relmat[:, qt:qt + 1], relcols, axis=mybir.AxisListType.X,
                    op=OP.add,
                )
                next(qkv, None)
            for _ in qkv:
                pass

            nc.sync.dma_start(
                out=rel.rearrange("(t p) -> p t", p=128), in_=relmat
            )
    nc.compile()
    return nc


def build_lb(S=SEQ, H=HIDDEN, NHC=NUM_HEADS // N_CORES, HD=HEAD_DIM,
             window=LOCAL_WINDOW):
    """Per-core (attention heads): partial (S, H) f16 = softmax-attn @ Wo rows."""
    nc = bacc.Bacc("TRN2", target_bir_lowering=False, debug=False)
    KC, NQ, QT, OCC = S // 128, S // 512, S // 128, H // 512
    qTh = nc.dram_tensor("qTh", [NHC * HD, S], F16, kind="ExternalInput")
    kTh = nc.dram_tensor("kTh", [NHC * HD, S], F16, kind="ExternalInput")
    vh = nc.dram_tensor("vh", [S, NHC * HD], F16, kind="ExternalInput")
    vslh = nc.dram_tensor("vslh", [S, NHC * HD], F16, kind="ExternalInput")
    woh = nc.dram_tensor("woh", [NHC * HD, H], F16, kind="ExternalInput")
    hivec = nc.dram_tensor("hivec", [S], F16, kind="ExternalInput")
    selv = nc.dram_tensor("selv", [S], F16, kind="ExternalInput")
    part = nc.dram_tensor("part", [S, H], F16, kind="ExternalOutput")

    scale = 1.0 / math.sqrt(HD)
    AF = mybir.ActivationFunctionType
    OP = mybir.AluOpType

    with TileContext(nc) as tc:
        with (
            tc.tile_pool(name="const", bufs=1) as cpool,
            tc.tile_pool(name="qk", bufs=1) as qkpool,
            tc.tile_pool(name="vv", bufs=1) as vpool,
            tc.tile_pool(name="et", bufs=2) as etpool,
            tc.tile_pool(name="aon", bufs=1) as aopool,
            tc.tile_pool(name="dr", bufs=2) as drpool,
            tc.tile_pool(name="ev", bufs=2) as evpool,
            tc.tile_pool(name="ps", bufs=1, space="PSUM") as pspool,
        ):
            # head-0 k/q first (the first score matmuls' operands), then v,
            # THEN the hvec/svec scatter loads -- those are 2048 tiny
            # descriptors each, and their generation ahead of q/k delays PE
            # start; they aren't read until the first masks (~2us later).
            qsb, ksb, vhf, vsl = [], [], [], []
            for h in range(NHC):
                k = qkpool.tile([128, S], F16, name=f"ksb{h}")
                nc.sync.dma_start(out=k, in_=kTh[h * HD:(h + 1) * HD, :])
                ksb.append(k)
                q = qkpool.tile([128, S], F16, name=f"qsb{h}")
                nc.sync.dma_start(out=q, in_=qTh[h * HD:(h + 1) * HD, :])
                qsb.append(q)
                vt = vpool.tile([128, KC * HD], F16, name=f"vhf{h}")
                nc.sync.dma_start(
                    out=vt.rearrange("p (t d) -> p t d", t=KC),
                    in_=vh[:, h * HD:(h + 1) * HD].rearrange(
                        "(t p) d -> p t d", p=128),
                )
                vhf.append(vt)
                vs = vpool.tile([128, KC * HD], F16, name=f"vsl{h}")
                nc.sync.dma_start(
                    out=vs.rearrange("p (t d) -> p t d", t=KC),
                    in_=vslh[:, h * HD:(h + 1) * HD].rearrange(
                        "(t p) d -> p t d", p=128),
                )
                vsl.append(vs)
                if h == 0:
                    hvec = cpool.tile([128, KC], F16, name="hvec")
                    nc.sync.dma_start(
                        out=hvec, in_=hivec.rearrange("(t p) -> p t", p=128))
                    svec = cpool.tile([128, KC], F16, name="svec")
                    nc.sync.dma_start(
                        out=svec, in_=selv.rearrange("(t p) -> p t", p=128))

            iota = cpool.tile([128, S], F16, name="iota")
            nc.gpsimd.iota(
                iota, pattern=[[1, S]], base=0, channel_multiplier=0,
                allow_small_or_imprecise_dtypes=True,
            )
            ones = cpool.tile([128, 1], F16, name="ones")
            nc.vector.memset(ones, 1.0)

            wsb = []
            for h in range(NHC):
                w = qkpool.tile([128, H], F16, name=f"wsb{h}")
                nc.sync.dma_start(out=w, in_=woh[h * HD:(h + 1) * HD, :])
                wsb.append(w)

            aon = [aopool.tile([128, S], F16, name=f"aon{h}")
                   for h in range(NHC)]

            def make_norm(h, avp, den128):
                def emit_norm(qc):
                    # den -> reciprocal -> partition-broadcast -> normalize;
                    # DVE/gpsimd only, so PE never waits on this chain except
                    # through the av/den bank reuse semaphores.
                    q0 = qc * 512
                    dq = drpool.tile([1, 512], F32, tag=f"dq{qc}",
                                     name=f"dq{qc}")
                    nc.vector.tensor_copy(dq, den128[32 * qc:32 * qc + 1, :])
                    rq = drpool.tile([1, 512], F32, tag=f"rq{qc}",
                                     name=f"rq{qc}")
                    rs = drpool.tile([1, 512], F32, tag=f"rs{qc}",
                                     name=f"rs{qc}")
                    nc.vector.reciprocal_approx_accurate(rq, dq, rs)
                    rbs = drpool.tile([128, 512], F32, tag="rbs", name="rbs")
                    nc.gpsimd.partition_broadcast(rbs, rq)
                    nc.vector.scalar_tensor_tensor(
                        aon[h][:, q0:q0 + 512], rbs, 1.0, avp[qc],
                        op0=OP.mult, op1=OP.mult,
                    )
                return emit_norm

            for h in range(NHC):
                avp = [
                    pspool.tile([128, 512], F32, tag=f"av{qc}", bufs=1,
                                name=f"av{qc}")
                    for qc in range(NQ)
                ]
                den128 = pspool.tile([128, 512], F32, tag="den", bufs=1,
                                     name="den128")
                emit_norm = make_norm(h, avp, den128)

                def emit_av_den(kc, qcs, far, ets):
                    for qc in qcs:
                        lhs_av = vsl[h] if far[qc] else vhf[h]
                        nc.tensor.matmul(
                            avp[qc], lhs_av[:, kc * 128:(kc + 1) * 128],
                            ets[qc], start=(kc == 0),
                            stop=(kc == (qc * 512 + 511) // 128),
                        )
                    for qc in qcs:
                        lhs_den = svec[:, kc:kc + 1] if far[qc] else ones
                        nc.tensor.matmul(
                            den128[32 * qc:32 * qc + 1, :], lhs_den, ets[qc],
                            start=(kc == 0),
                            stop=(kc == (qc * 512 + 511) // 128),
                            tile_position=(0, 32 * qc),
                        )
                    # a q-chunk whose last key tile just finished can be
                    # normalized now, overlapping the remaining kc loop
                    for qc in qcs:
                        if kc == (qc * 512 + 511) // 128:
                            emit_norm(qc)

                pend = None
                for kc in range(KC):
                    k0 = kc * 128
                    qcs = [qc for qc in range(NQ) if qc * 512 + 511 >= k0]
                    far = {qc: qc * 512 > k0 + 127 + window for qc in qcs}
                    ets = {}
                    for qc in qcs:
                        q0 = qc * 512
                        sps = pspool.tile([128, 512], F32, tag="sc", bufs=3,
                                          name="sps")
                        nc.tensor.matmul(
                            sps, ksb[h][:, k0:k0 + 128],
                            qsb[h][:, q0:q0 + 512], start=True, stop=True,
                        )
                        et = etpool.tile([128, 512], F16, tag=f"et{qc}",
                                         name=f"et{qc}")
                        ets[qc] = et
                        nc.scalar.activation(et, sps, AF.Exp, scale=scale)
                        if far[qc]:
                            continue  # sel-mask folded into vsl/svec operands
                        if q0 < k0 + 128:
                            # causal: zero where q < k
                            nc.gpsimd.affine_select(
                                out=et, in_=et, compare_op=OP.is_ge, fill=0.0,
                                base=q0 - k0, channel_multiplier=-1,
                                pattern=[[1, 512]],
                            )
                        if q0 + 511 > k0 + window:
                            nc.vector.scalar_tensor_tensor(
                                et, iota[:, q0:q0 + 512], hvec[:, kc:kc + 1],
                                et, op0=OP.is_le, op1=OP.mult,
                            )
                    if pend is not None:
                        emit_av_den(*pend)
                    pend = (kc, qcs, far, ets)
                emit_av_den(*pend)

            # output projection: partial = sum_h aon_h @ Wo rows.
            # oc=3 uses the den bank so no wops waits on the (late) av3
            # normalize read; qt order is free since all norms are done.
            wop_tags = ["av0", "av1", "av2", "den"]
            nev = 0
            for qt in range(QT):
                wops = [
                    pspool.tile([128, 512], F32, tag=wop_tags[oc], bufs=1,
                                name=f"wops{oc}")
                    for oc in range(OCC)
                ]
                for h in range(NHC):
                    for oc in range(OCC):
                        nc.tensor.matmul(
                            wops[oc], aon[h][:, qt * 128:(qt + 1) * 128],
                            wsb[h][:, oc * 512:(oc + 1) * 512],
                            start=(h == 0), stop=(h == NHC - 1),
                        )
                for oc in range(OCC):
                    ot = evpool.tile([128, 512], F16, tag=f"ot{oc}",
                                     name=f"ot{oc}")
                    nev += 1
                    if nev % 2 == 0:
                        nc.scalar.copy(ot, wops[oc])
                    else:
                        nc.vector.tensor_copy(ot, wops[oc])
                    nc.sync.dma_start(
                        out=part[qt * 128:(qt + 1) * 128,
                                 oc * 512:(oc + 1) * 512],
                        in_=ot,
                    )
    nc.compile()
    return nc


_CACHE = {}


def _get(name, builder, *args):
    key = (name,) + args
    if key not in _CACHE:
        _CACHE[key] = builder(*args)
    return _CACHE[key]


def _run(nc, in_maps):
    res = run_bass_kernel_spmd(
        nc, in_maps, core_ids=list(range(N_CORES)), trace=_TRACE["on"]
    )
    if _TRACE["on"] and res.exec_time_ns is not None:
        _TRACE["exec_ns"].append(res.exec_time_ns)
    return res.results


def kernel(hidden_states, Wq, Wk, Wv, Wo, Wq_ind, Wk_ind, head_weights,
           temperature_param):
    hidden_states = np.asarray(hidden_states, dtype=FP32)
    Wq, Wk, Wv, Wo = (np.asarray(a, dtype=FP32) for a in (Wq, Wk, Wv, Wo))
    Wq_ind = np.asarray(Wq_ind, dtype=FP32)
    Wk_ind = np.asarray(Wk_ind, dtype=FP32)
    head_weights = np.asarray(head_weights, dtype=FP32)

    B, S, H = hidden_states.shape
    assert B == 1 and H == HIDDEN and S == SEQ
    CS = H // N_CORES

    # fused indexer weights: qp = q_lin@Wq_ind = hidden@(Wq@Wq_ind)
    Wqi_f = Wq @ Wq_ind
    Wki_f = Wk @ Wk_ind

    hidT = np.ascontiguousarray(hidden_states[0].T).astype(np.float16)

    # ---- launch A: projections + indexer rel ----
    nca = _get("la", build_la, S, H, CS)
    ina = [
        {
            "hidT": hidT,
            "wq": np.ascontiguousarray(Wq[:, c * CS:(c + 1) * CS]).astype(np.float16),
            "wk": np.ascontiguousarray(Wk[:, c * CS:(c + 1) * CS]).astype(np.float16),
            "wv": np.ascontiguousarray(Wv[:, c * CS:(c + 1) * CS]).astype(np.float16),
            "wqi": np.ascontiguousarray(Wqi_f[:, c * CS:(c + 1) * CS]).astype(np.float16),
            "wki": np.ascontiguousarray(Wki_f[:, c * CS:(c + 1) * CS]).astype(np.float16),
        }
        for c in range(N_CORES)
    ]
    ra = _run(nca, ina)

    rel = np.zeros(S, dtype=np.float64)
    for c in range(N_CORES):
        rel += float(head_weights[c]) * np.asarray(ra[c]["rel"], dtype=np.float64)
    # exp(-temp) scaling is monotone; irrelevant for top-k selection.

    k_sel = min(MAX_SELECTED, S)
    top_idx = np.argpartition(-rel, k_sel - 1)[:k_sel]
    selected = np.zeros(S, dtype=bool)
    selected[top_idx] = True

    # ---- launch B: masked attention + output projection ----
    BIG = float(2 * S + 1024)
    hi = np.where(selected, BIG, np.arange(S, dtype=np.float64) + LOCAL_WINDOW)
    hi = hi.astype(np.float16)
    selv = selected.astype(np.float16)
    NHC = NUM_HEADS // N_CORES
    RW = NHC * HEAD_DIM

    ncb = _get("lb", build_lb, S, H, NHC, HEAD_DIM, LOCAL_WINDOW)
    inb = []
    for c in range(N_CORES):
        vhc = np.ascontiguousarray(
            np.asarray(ra[c]["vT"], dtype=np.float16).T)  # (S, 256) key-major
        inb.append({
            "qTh": np.asarray(ra[c]["qT"]),
            "kTh": np.asarray(ra[c]["kT"]),
            "vh": vhc,
            "vslh": np.ascontiguousarray(vhc * selv[:, None]),
            "woh": np.ascontiguousarray(Wo[c * RW:(c + 1) * RW]).astype(np.float16),
            "hivec": hi,
            "selv": selv,
        })
    rb = _run(ncb, inb)
    out = np.zeros((S, H), dtype=np.float32)
    for c in range(N_CORES):
        out += np.asarray(rb[c]["part"], dtype=np.float32)
    return out.reshape(B, S, H).astype(np.float32)


# revision 15
# speedup vs baseline: 1.0179x; 1.0149x over previous
"""DeepSeek sparse attention on 8 Trainium2 NeuronCores (Bass/Tile).

Two SPMD launches (down from three):

  A   (column/indexer-head-parallel): core c computes the 256-col slices
      of the q/k/v projections (emitted transposed, bf16/f16) AND its
      indexer head's relevance scores rel_c using HOST-FUSED indexer
      weights (Wq@Wq_ind, Wk@Wk_ind).  The fusion decouples the indexer
      from q_lin/k_lin, killing the baseline's launch 2 (which reloaded
      32MB/core of qT/kT).  All matmul inputs fp16 (same PE rate as
      f32r, half the DMA of f32, and 8x finer mantissa than bf16 --
      bf16-level noise flips borderline top-k keys, each flip costing
      ~1e-2 rel err).  PE order: indexer projections first, then indexer
      score groups interleaved into the q/k/v matmul stream so ACT relu
      latency never stalls PE.
  host: rel = sum_c w_c*rel_c; top-1024 -> selected mask; v transposed
      to key-major f16 and premultiplied by the mask; hi threshold vec.
  B   (attention-head-parallel): core c owns heads 2c, 2c+1: softmax
      attention with causal/local/selected masking + output-projection
      partial (f16).  PE issue order software-pipelined: scores of key
      tile kc+1 are issued before AV of kc, hiding exp/mask latency.
  host: out = sum_c partial_c.
"""

import math

import numpy as np

import concourse.bass as bass
import concourse.mybir as mybir
from concourse import bacc
from concourse.tile import TileContext
from concourse.bass_utils import run_bass_kernel_spmd

# Problem constants (hardcoded per contract)
HIDDEN = 2048
NUM_HEADS = 16
HEAD_DIM = 128
NUM_IND_HEADS = 8
IND_DIM = HIDDEN // NUM_IND_HEADS  # 256
MAX_SELECTED = 1024
LOCAL_WINDOW = 512
N_CORES = 8
SEQ = 2048

F32 = mybir.dt.float32
F32R = mybir.dt.float32r
F16 = mybir.dt.float16
FP32 = np.float32

_TRACE = {"on": False, "exec_ns": []}


def build_la(S=SEQ, H=HIDDEN, CS=HIDDEN // N_CORES):
    """Per-core: qT/kT/vT (CS, S) slices + indexer-head rel (S)."""
    nc = bacc.Bacc("TRN2", target_bir_lowering=False, debug=False)
    HT, MC, NQ, QT, DC = H // 128, CS // 128, S // 512, S // 128, IND_DIM // 128
    hidT = nc.dram_tensor("hidT", [H, S], F16, kind="ExternalInput")
    wq = nc.dram_tensor("wq", [H, CS], F16, kind="ExternalInput")
    wk = nc.dram_tensor("wk", [H, CS], F16, kind="ExternalInput")
    wv = nc.dram_tensor("wv", [H, CS], F16, kind="ExternalInput")
    wqi = nc.dram_tensor("wqi", [H, CS], F16, kind="ExternalInput")
    wki = nc.dram_tensor("wki", [H, CS], F16, kind="ExternalInput")
    qT = nc.dram_tensor("qT", [CS, S], F16, kind="ExternalOutput")
    kT = nc.dram_tensor("kT", [CS, S], F16, kind="ExternalOutput")
    vT = nc.dram_tensor("vT", [CS, S], F16, kind="ExternalOutput")
    rel = nc.dram_tensor("rel", [S], F32, kind="ExternalOutput")

    AF = mybir.ActivationFunctionType
    OP = mybir.AluOpType

    with TileContext(nc) as tc:
        with (
            tc.tile_pool(name="hid", bufs=1) as hpool,
            tc.tile_pool(name="wt", bufs=1) as wpool,
            tc.tile_pool(name="proj", bufs=1) as ppool,
            tc.tile_pool(name="ev", bufs=2) as opool,
            tc.tile_pool(name="scr", bufs=2) as scpool,
            tc.tile_pool(name="rc", bufs=2) as rcpool,
            tc.tile_pool(name="rm", bufs=1) as rmpool,
            tc.tile_pool(name="ps", bufs=1, space="PSUM") as pspool,
        ):
            def load_w(wdram):
                wr = wpool.tile([128, HT * CS], F16, name=f"w_{wdram.name}")
                nc.sync.dma_start(
                    out=wr.rearrange("p (t c) -> p t c", t=HT),
                    in_=wdram.rearrange("(t p) c -> p t c", p=128),
                )
                return wr

            def load_strip(t):
                hs = hpool.tile([128, S], F16, name=f"hid{t}")
                nc.sync.dma_start(out=hs, in_=hidT[t * 128:(t + 1) * 128, :])
                return hs

            # DMA order: the first matmul needs only wqi strip 0 (64KB) and
            # hid strip 0, so issue those two tiny/medium DMAs before the
            # bulk wqi load -- descriptor generation on the sync queue is
            # serial, and a big strided DMA ahead of strip 0 delays PE start.
            wqi_t = wpool.tile([128, HT * CS], F16, name="w_wqi")
            nc.sync.dma_start(out=wqi_t[:, 0:CS], in_=wqi[0:128, :])
            hids = [load_strip(0)]
            nc.sync.dma_start(
                out=wqi_t[:, CS:].rearrange("p (t c) -> p t c", t=HT - 1),
                in_=wqi[128:, :].rearrange("(t p) c -> p t c", p=128),
            )
            wki_t = load_w(wki)
            hids += [load_strip(t) for t in range(1, HT)]
            wq_t, wk_t, wv_t = load_w(wq), load_w(wk), load_w(wv)

            qpi = [ppool.tile([128, S], F16, name=f"qpi{d}") for d in range(DC)]
            kpi = [ppool.tile([128, S], F16, name=f"kpi{d}") for d in range(DC)]

            # ---- phase 1: indexer projections (hidden @ fused weights) ----
            for wt, dst in ((wqi_t, qpi), (wki_t, kpi)):
                for mc in range(MC):
                    psums = [
                        pspool.tile([128, 512], F32, tag=f"p{qc}", name=f"p{qc}")
                        for qc in range(NQ)
                    ]
                    for t in range(HT):
                        lhsT = wt[:, t * CS + mc * 128: t * CS + mc * 128 + 128]
                        for qc in range(NQ):
                            nc.tensor.matmul(
                                psums[qc], lhsT,
                                hids[t][:, qc * 512:(qc + 1) * 512],
                                start=(t == 0), stop=(t == HT - 1),
                            )
                    for qc in range(NQ):
                        nc.vector.tensor_copy(
                            dst[mc][:, qc * 512:(qc + 1) * 512], psums[qc]
                        )

            # ---- phase 2: q/k/v projections interleaved with score groups ----
            relmat = rmpool.tile([128, QT], F32, name="relmat")

            def gen_qkv():
                for wt, odram, odt in (
                    (wq_t, qT, F16), (wk_t, kT, F16), (wv_t, vT, F16),
                ):
                    for mc in range(MC):
                        psums = [
                            pspool.tile([128, 512], F32, tag=f"p{qc}",
                                        name=f"pp{qc}")
                            for qc in range(NQ)
                        ]
                        for t in range(HT):
                            lhsT = wt[:, t * CS + mc * 128:
                                      t * CS + mc * 128 + 128]
                            for qc in range(NQ):
                                nc.tensor.matmul(
                                    psums[qc], lhsT,
                                    hids[t][:, qc * 512:(qc + 1) * 512],
                                    start=(t == 0), stop=(t == HT - 1),
                                )
                            if t == 7:
                                yield
                        for qc in range(NQ):
                            ot = opool.tile([128, 512], odt, tag=f"ot{qc}",
                                            name=f"ot{qc}")
                            nc.vector.tensor_copy(ot, psums[qc])
                            nc.sync.dma_start(
                                out=odram[mc * 128:(mc + 1) * 128,
                                          qc * 512:(qc + 1) * 512],
                                in_=ot,
                            )
                        yield

            qkv = gen_qkv()
            for qt in range(QT):
                # score group qt: rel rows for q-tile qt
                sps = [
                    pspool.tile([128, 512], F32, tag=f"s{kc}", name=f"s{kc}")
                    for kc in range(NQ)
                ]
                for kc in range(NQ):
                    for d in range(DC):
                        nc.tensor.matmul(
                            sps[kc],
                            qpi[d][:, qt * 128:(qt + 1) * 128],
                            kpi[d][:, kc * 512:(kc + 1) * 512],
                            start=(d == 0), stop=(d == DC - 1),
                        )
                relcols = rcpool.tile([128, NQ], F32, tag="relcols",
                                      name="relcols")
                for kc in range(NQ):
                    scratch = scpool.tile([128, 512], F16, tag="scratch",
                                          name="scratch")
                    nc.scalar.activation(
                        scratch, sps[kc], AF.Relu,
                        accum_out=relcols[:, kc:kc + 1],
                    )
                nc.vector.tensor_reduce(
                    relmat[:, qt:qt + 1], relcols, axis=mybir.AxisListType.X,
                    op=OP.add,
                )
                next(qkv, None)
            for _ in qkv:
                pass

            nc.sync.dma_start(
                out=rel.rearrange("(t p) -> p t", p=128), in_=relmat
            )
    nc.compile()
    return nc


def build_lb(S=SEQ, H=HIDDEN, NHC=NUM_HEADS // N_CORES, HD=HEAD_DIM,
             window=LOCAL_WINDOW):
    """Per-core (attention heads): partial (S, H) f16 = softmax-attn @ Wo rows."""
    nc = bacc.Bacc("TRN2", target_bir_lowering=False, debug=False)
    KC, NQ, QT, OCC = S // 128, S // 512, S // 128, H // 512
    qTh = nc.dram_tensor("qTh", [NHC * HD, S], F16, kind="ExternalInput")
    kTh = nc.dram_tensor("kTh", [NHC * HD, S], F16, kind="ExternalInput")
    vh = nc.dram_tensor("vh", [S, NHC * HD], F16, kind="ExternalInput")
    vslh = nc.dram_tensor("vslh", [S, NHC * HD], F16, kind="ExternalInput")
    woh = nc.dram_tensor("woh", [NHC * HD, H], F16, kind="ExternalInput")
    hivec = nc.dram_tensor("hivec", [S], F16, kind="ExternalInput")
    selv = nc.dram_tensor("selv", [S], F16, kind="ExternalInput")
    part = nc.dram_tensor("part", [S, H], F16, kind="ExternalOutput")

    scale = 1.0 / math.sqrt(HD)
    AF = mybir.ActivationFunctionType
    OP = mybir.AluOpType

    with TileContext(nc) as tc:
        with (
            tc.tile_pool(name="const", bufs=1) as cpool,
            tc.tile_pool(name="qk", bufs=1) as qkpool,
            tc.tile_pool(name="vv", bufs=1) as vpool,
            tc.tile_pool(name="et", bufs=2) as etpool,
            tc.tile_pool(name="aon", bufs=1) as aopool,
            tc.tile_pool(name="dr", bufs=2) as drpool,
            tc.tile_pool(name="ev", bufs=2) as evpool,
            tc.tile_pool(name="ps", bufs=1, space="PSUM") as pspool,
        ):
            # head-0 k/q first (the first score matmuls' operands), then v,
            # THEN the hvec/svec scatter loads -- those are 2048 tiny
            # descriptors each, and their generation ahead of q/k delays PE
            # start; they aren't read until the first masks (~2us later).
            qsb, ksb, vhf, vsl = [], [], [], []
            for h in range(NHC):
                k = qkpool.tile([128, S], F16, name=f"ksb{h}")
                nc.sync.dma_start(out=k, in_=kTh[h * HD:(h + 1) * HD, :])
                ksb.append(k)
                q = qkpool.tile([128, S], F16, name=f"qsb{h}")
                nc.sync.dma_start(out=q, in_=qTh[h * HD:(h + 1) * HD, :])
                qsb.append(q)
                vt = vpool.tile([128, KC * HD], F16, name=f"vhf{h}")
                nc.sync.dma_start(
                    out=vt.rearrange("p (t d) -> p t d", t=KC),
                    in_=vh[:, h * HD:(h + 1) * HD].rearrange(
                        "(t p) d -> p t d", p=128),
                )
                vhf.append(vt)
                vs = vpool.tile([128, KC * HD], F16, name=f"vsl{h}")
                nc.sync.dma_start(
                    out=vs.rearrange("p (t d) -> p t d", t=KC),
                    in_=vslh[:, h * HD:(h + 1) * HD].rearrange(
                        "(t p) d -> p t d", p=128),
                )
                vsl.append(vs)
                if h == 0:
                    hvec = cpool.tile([128, KC], F16, name="hvec")
                    nc.sync.dma_start(
                        out=hvec, in_=hivec.rearrange("(t p) -> p t", p=128))
                    svec = cpool.tile([128, KC], F16, name="svec")
                    nc.sync.dma_start(
                        out=svec, in_=selv.rearrange("(t p) -> p t", p=128))

            iota = cpool.tile([128, S], F16, name="iota")
            nc.gpsimd.iota(
                iota, pattern=[[1, S]], base=0, channel_multiplier=0,
                allow_small_or_imprecise_dtypes=True,
            )
            ones = cpool.tile([128, 1], F16, name="ones")
            nc.vector.memset(ones, 1.0)

            wsb = []
            for h in range(NHC):
                w = qkpool.tile([128, H], F16, name=f"wsb{h}")
                nc.sync.dma_start(out=w, in_=woh[h * HD:(h + 1) * HD, :])
                wsb.append(w)

            aon = [aopool.tile([128, S], F16, name=f"aon{h}")
                   for h in range(NHC)]

            def make_norm(h, avp, den128):
                def emit_norm(qc):
                    # den -> reciprocal -> partition-broadcast -> normalize;
                    # DVE/gpsimd only, so PE never waits on this chain except
                    # through the av/den bank reuse semaphores.
                    q0 = qc * 512
                    dq = drpool.tile([1, 512], F32, tag=f"dq{qc}",
                                     name=f"dq{qc}")
                    nc.vector.tensor_copy(dq, den128[32 * qc:32 * qc + 1, :])
                    rq = drpool.tile([1, 512], F32, tag=f"rq{qc}",
                                     name=f"rq{qc}")
                    rs = drpool.tile([1, 512], F32, tag=f"rs{qc}",
                                     name=f"rs{qc}")
                    nc.vector.reciprocal_approx_accurate(rq, dq, rs)
                    rbs = drpool.tile([128, 512], F32, tag="rbs", name="rbs")
                    nc.gpsimd.partition_broadcast(rbs, rq)
                    nc.vector.scalar_tensor_tensor(
                        aon[h][:, q0:q0 + 512], rbs, 1.0, avp[qc],
                        op0=OP.mult, op1=OP.mult,
                    )
                return emit_norm

            for h in range(NHC):
                avp = [
                    pspool.tile([128, 512], F32, tag=f"av{qc}", bufs=1,
                                name=f"av{qc}")
                    for qc in range(NQ)
                ]
                den128 = pspool.tile([128, 512], F32, tag="den", bufs=1,
                                     name="den128")
                emit_norm = make_norm(h, avp, den128)

                def emit_av_den(kc, qcs, far, ets):
                    for qc in qcs:
                        lhs_av = vsl[h] if far[qc] else vhf[h]
                        nc.tensor.matmul(
                            avp[qc], lhs_av[:, kc * 128:(kc + 1) * 128],
                            ets[qc], start=(kc == 0),
                            stop=(kc == (qc * 512 + 511) // 128),
                        )
                    for qc in qcs:
                        lhs_den = svec[:, kc:kc + 1] if far[qc] else ones
                        nc.tensor.matmul(
                            den128[32 * qc:32 * qc + 1, :], lhs_den, ets[qc],
                            start=(kc == 0),
                            stop=(kc == (qc * 512 + 511) // 128),
                            tile_position=(0, 32 * qc),
                        )
                    # a q-chunk whose last key tile just finished can be
                    # normalized now, overlapping the remaining kc loop
                    for qc in qcs:
                        if kc == (qc * 512 + 511) // 128:
                            emit_norm(qc)

                pend = None
                for kc in range(KC):
                    k0 = kc * 128
                    qcs = [qc for qc in range(NQ) if qc * 512 + 511 >= k0]
                    far = {qc: qc * 512 > k0 + 127 + window for qc in qcs}
                    ets = {}
                    for qc in qcs:
                        q0 = qc * 512
                        sps = pspool.tile([128, 512], F32, tag="sc", bufs=3,
                                          name="sps")
                        nc.tensor.matmul(
                            sps, ksb[h][:, k0:k0 + 128],
                            qsb[h][:, q0:q0 + 512], start=True, stop=True,
                        )
                        et = etpool.tile([128, 512], F16, tag=f"et{qc}",
                                         name=f"et{qc}")
                        ets[qc] = et
                        diag = not far[qc] and q0 < k0 + 128
                        # on the diagonal tile, cols [q0, k0) are entirely
                        # causally masked: skip their exp (ACT is the tight
                        # engine here); affine_select writes 0 there anyway.
                        pre = k0 - q0 if diag else 0
                        nc.scalar.activation(
                            et[:, pre:], sps[:, pre:], AF.Exp, scale=scale)
                        if far[qc]:
                            continue  # sel-mask folded into vsl/svec operands
                        if diag:
                            # causal: zero where q < k (fills the skipped
                            # prefix too)
                            nc.gpsimd.affine_select(
                                out=et, in_=et, compare_op=OP.is_ge, fill=0.0,
                                base=q0 - k0, channel_multiplier=-1,
                                pattern=[[1, 512]],
                            )
                        if q0 + 511 > k0 + window:
                            nc.vector.scalar_tensor_tensor(
                                et, iota[:, q0:q0 + 512], hvec[:, kc:kc + 1],
                                et, op0=OP.is_le, op1=OP.mult,
                            )
                    if pend is not None:
                        emit_av_den(*pend)
                    pend = (kc, qcs, far, ets)
                emit_av_den(*pend)

            # output projection: partial = sum_h aon_h @ Wo rows.
            # oc=3 uses the den bank so no wops waits on the (late) av3
            # normalize read; qt order is free since all norms are done.
            wop_tags = ["av0", "av1", "av2", "den"]
            nev = 0
            for qt in range(QT):
                wops = [
                    pspool.tile([128, 512], F32, tag=wop_tags[oc], bufs=1,
                                name=f"wops{oc}")
                    for oc in range(OCC)
                ]
                for h in range(NHC):
                    for oc in range(OCC):
                        nc.tensor.matmul(
                            wops[oc], aon[h][:, qt * 128:(qt + 1) * 128],
                            wsb[h][:, oc * 512:(oc + 1) * 512],
                            start=(h == 0), stop=(h == NHC - 1),
                        )
                for oc in range(OCC):
                    ot = evpool.tile([128, 512], F16, tag=f"ot{oc}",
                                     name=f"ot{oc}")
                    nev += 1
                    if nev % 2 == 0:
                        nc.scalar.copy(ot, wops[oc])
                    else:
                        nc.vector.tensor_copy(ot, wops[oc])
                    nc.sync.dma_start(
                        out=part[qt * 128:(qt + 1) * 128,
                                 oc * 512:(oc + 1) * 512],
                        in_=ot,
                    )
    nc.compile()
    return nc


_CACHE = {}


def _get(name, builder, *args):
    key = (name,) + args
    if key not in _CACHE:
        _CACHE[key] = builder(*args)
    return _CACHE[key]


def _run(nc, in_maps):
    res = run_bass_kernel_spmd(
        nc, in_maps, core_ids=list(range(N_CORES)), trace=_TRACE["on"]
    )
    if _TRACE["on"] and res.exec_time_ns is not None:
        _TRACE["exec_ns"].append(res.exec_time_ns)
    return res.results


def kernel(hidden_states, Wq, Wk, Wv, Wo, Wq_ind, Wk_ind, head_weights,
           temperature_param):
    hidden_states = np.asarray(hidden_states, dtype=FP32)
    Wq, Wk, Wv, Wo = (np.asarray(a, dtype=FP32) for a in (Wq, Wk, Wv, Wo))
    Wq_ind = np.asarray(Wq_ind, dtype=FP32)
    Wk_ind = np.asarray(Wk_ind, dtype=FP32)
    head_weights = np.asarray(head_weights, dtype=FP32)

    B, S, H = hidden_states.shape
    assert B == 1 and H == HIDDEN and S == SEQ
    CS = H // N_CORES

    # fused indexer weights: qp = q_lin@Wq_ind = hidden@(Wq@Wq_ind)
    Wqi_f = Wq @ Wq_ind
    Wki_f = Wk @ Wk_ind

    hidT = np.ascontiguousarray(hidden_states[0].T).astype(np.float16)

    # ---- launch A: projections + indexer rel ----
    nca = _get("la", build_la, S, H, CS)
    ina = [
        {
            "hidT": hidT,
            "wq": np.ascontiguousarray(Wq[:, c * CS:(c + 1) * CS]).astype(np.float16),
            "wk": np.ascontiguousarray(Wk[:, c * CS:(c + 1) * CS]).astype(np.float16),
            "wv": np.ascontiguousarray(Wv[:, c * CS:(c + 1) * CS]).astype(np.float16),
            "wqi": np.ascontiguousarray(Wqi_f[:, c * CS:(c + 1) * CS]).astype(np.float16),
            "wki": np.ascontiguousarray(Wki_f[:, c * CS:(c + 1) * CS]).astype(np.float16),
        }
        for c in range(N_CORES)
    ]
    ra = _run(nca, ina)

    rel = np.zeros(S, dtype=np.float64)
    for c in range(N_CORES):
        rel += float(head_weights[c]) * np.asarray(ra[c]["rel"], dtype=np.float64)
    # exp(-temp) scaling is monotone; irrelevant for top-k selection.

    k_sel = min(MAX_SELECTED, S)
    top_idx = np.argpartition(-rel, k_sel - 1)[:k_sel]
    selected = np.zeros(S, dtype=bool)
    selected[top_idx] = True

    # ---- launch B: masked attention + output projection ----
    BIG = float(2 * S + 1024)
    hi = np.where(selected, BIG, np.arange(S, dtype=np.float64) + LOCAL_WINDOW)
    hi = hi.astype(np.float16)
    selv = selected.astype(np.float16)
    NHC = NUM_HEADS // N_CORES
    RW = NHC * HEAD_DIM

    ncb = _get("lb", build_lb, S, H, NHC, HEAD_DIM, LOCAL_WINDOW)
    inb = []
    for c in range(N_CORES):
        vhc = np.ascontiguousarray(
            np.asarray(ra[c]["vT"], dtype=np.float16).T)  # (S, 256) key-major
        inb.append({
            "qTh": np.asarray(ra[c]["qT"]),
            "kTh": np.asarray(ra[c]["kT"]),
            "vh": vhc,
            "vslh": np.ascontiguousarray(vhc * selv[:, None]),
            "woh": np.ascontiguousarray(Wo[c * RW:(c + 1) * RW]).astype(np.float16),
            "hivec": hi,
            "selv": selv,
        })
    rb = _run(ncb, inb)
    out = np.zeros((S, H), dtype=np.float32)
    for c in range(N_CORES):
        out += np.asarray(rb[c]["part"], dtype=np.float32)
    return out.reshape(B, S, H).astype(np.float32)


# revision 16
# speedup vs baseline: 1.0381x; 1.0198x over previous
"""DeepSeek sparse attention on 8 Trainium2 NeuronCores (Bass/Tile).

Two SPMD launches (down from three):

  A   (column/indexer-head-parallel): core c computes the 256-col slices
      of the q/k/v projections (emitted transposed, bf16/f16) AND its
      indexer head's relevance scores rel_c using HOST-FUSED indexer
      weights (Wq@Wq_ind, Wk@Wk_ind).  The fusion decouples the indexer
      from q_lin/k_lin, killing the baseline's launch 2 (which reloaded
      32MB/core of qT/kT).  All matmul inputs fp16 (same PE rate as
      f32r, half the DMA of f32, and 8x finer mantissa than bf16 --
      bf16-level noise flips borderline top-k keys, each flip costing
      ~1e-2 rel err).  PE order: indexer projections first, then indexer
      score groups interleaved into the q/k/v matmul stream so ACT relu
      latency never stalls PE.
  host: rel = sum_c w_c*rel_c; top-1024 -> selected mask; v transposed
      to key-major f16 and premultiplied by the mask; hi threshold vec.
  B   (attention-head-parallel): core c owns heads 2c, 2c+1: softmax
      attention with causal/local/selected masking + output-projection
      partial (f16).  PE issue order software-pipelined: scores of key
      tile kc+1 are issued before AV of kc, hiding exp/mask latency.
  host: out = sum_c partial_c.
"""

import math

import numpy as np

import concourse.bass as bass
import concourse.mybir as mybir
from concourse import bacc
from concourse.tile import TileContext
from concourse.bass_utils import run_bass_kernel_spmd

# Problem constants (hardcoded per contract)
HIDDEN = 2048
NUM_HEADS = 16
HEAD_DIM = 128
NUM_IND_HEADS = 8
IND_DIM = HIDDEN // NUM_IND_HEADS  # 256
MAX_SELECTED = 1024
LOCAL_WINDOW = 512
N_CORES = 8
SEQ = 2048

F32 = mybir.dt.float32
F32R = mybir.dt.float32r
F16 = mybir.dt.float16
FP32 = np.float32

_TRACE = {"on": False, "exec_ns": []}


def build_la(S=SEQ, H=HIDDEN, CS=HIDDEN // N_CORES):
    """Per-core: qT/kT/vT (CS, S) slices + indexer-head rel (S)."""
    nc = bacc.Bacc("TRN2", target_bir_lowering=False, debug=False)
    HT, MC, NQ, QT, DC = H // 128, CS // 128, S // 512, S // 128, IND_DIM // 128
    hidT = nc.dram_tensor("hidT", [H, S], F16, kind="ExternalInput")
    wq = nc.dram_tensor("wq", [H, CS], F16, kind="ExternalInput")
    wk = nc.dram_tensor("wk", [H, CS], F16, kind="ExternalInput")
    wv = nc.dram_tensor("wv", [H, CS], F16, kind="ExternalInput")
    wqi = nc.dram_tensor("wqi", [H, CS], F16, kind="ExternalInput")
    wki = nc.dram_tensor("wki", [H, CS], F16, kind="ExternalInput")
    qT = nc.dram_tensor("qT", [CS, S], F16, kind="ExternalOutput")
    kT = nc.dram_tensor("kT", [CS, S], F16, kind="ExternalOutput")
    vT = nc.dram_tensor("vT", [CS, S], F16, kind="ExternalOutput")
    rel = nc.dram_tensor("rel", [S], F32, kind="ExternalOutput")

    AF = mybir.ActivationFunctionType
    OP = mybir.AluOpType

    with TileContext(nc) as tc:
        with (
            tc.tile_pool(name="hid", bufs=1) as hpool,
            tc.tile_pool(name="wt", bufs=1) as wpool,
            tc.tile_pool(name="proj", bufs=1) as ppool,
            tc.tile_pool(name="ev", bufs=2) as opool,
            tc.tile_pool(name="scr", bufs=2) as scpool,
            tc.tile_pool(name="rc", bufs=2) as rcpool,
            tc.tile_pool(name="rm", bufs=1) as rmpool,
            tc.tile_pool(name="ps", bufs=1, space="PSUM") as pspool,
        ):
            def load_w(wdram):
                wr = wpool.tile([128, HT * CS], F16, name=f"w_{wdram.name}")
                nc.sync.dma_start(
                    out=wr.rearrange("p (t c) -> p t c", t=HT),
                    in_=wdram.rearrange("(t p) c -> p t c", p=128),
                )
                return wr

            def load_strip(t):
                hs = hpool.tile([128, S], F16, name=f"hid{t}")
                nc.sync.dma_start(out=hs, in_=hidT[t * 128:(t + 1) * 128, :])
                return hs

            # DMA order: the first matmul needs only wqi strip 0 (64KB) and
            # hid strip 0, so issue those two tiny/medium DMAs before the
            # bulk wqi load -- descriptor generation on the sync queue is
            # serial, and a big strided DMA ahead of strip 0 delays PE start.
            wqi_t = wpool.tile([128, HT * CS], F16, name="w_wqi")
            nc.sync.dma_start(out=wqi_t[:, 0:CS], in_=wqi[0:128, :])
            hids = [load_strip(0)]
            nc.sync.dma_start(
                out=wqi_t[:, CS:].rearrange("p (t c) -> p t c", t=HT - 1),
                in_=wqi[128:, :].rearrange("(t p) c -> p t c", p=128),
            )
            wki_t = load_w(wki)
            hids += [load_strip(t) for t in range(1, HT)]
            wq_t, wk_t, wv_t = load_w(wq), load_w(wk), load_w(wv)

            qpi = [ppool.tile([128, S], F16, name=f"qpi{d}") for d in range(DC)]
            kpi = [ppool.tile([128, S], F16, name=f"kpi{d}") for d in range(DC)]

            # ---- phase 1: indexer projections (hidden @ fused weights) ----
            for wt, dst in ((wqi_t, qpi), (wki_t, kpi)):
                for mc in range(MC):
                    psums = [
                        pspool.tile([128, 512], F32, tag=f"p{qc}", name=f"p{qc}")
                        for qc in range(NQ)
                    ]
                    for t in range(HT):
                        lhsT = wt[:, t * CS + mc * 128: t * CS + mc * 128 + 128]
                        for qc in range(NQ):
                            nc.tensor.matmul(
                                psums[qc], lhsT,
                                hids[t][:, qc * 512:(qc + 1) * 512],
                                start=(t == 0), stop=(t == HT - 1),
                            )
                    for qc in range(NQ):
                        nc.vector.tensor_copy(
                            dst[mc][:, qc * 512:(qc + 1) * 512], psums[qc]
                        )

            # ---- phase 2: q/k/v projections interleaved with score groups ----
            relmat = rmpool.tile([128, QT], F32, name="relmat")

            def gen_qkv():
                for wt, odram, odt in (
                    (wq_t, qT, F16), (wk_t, kT, F16), (wv_t, vT, F16),
                ):
                    for mc in range(MC):
                        psums = [
                            pspool.tile([128, 512], F32, tag=f"p{qc}",
                                        name=f"pp{qc}")
                            for qc in range(NQ)
                        ]
                        for t in range(HT):
                            lhsT = wt[:, t * CS + mc * 128:
                                      t * CS + mc * 128 + 128]
                            for qc in range(NQ):
                                nc.tensor.matmul(
                                    psums[qc], lhsT,
                                    hids[t][:, qc * 512:(qc + 1) * 512],
                                    start=(t == 0), stop=(t == HT - 1),
                                )
                            if t == 7:
                                yield
                        for qc in range(NQ):
                            ot = opool.tile([128, 512], odt, tag=f"ot{qc}",
                                            name=f"ot{qc}")
                            nc.vector.tensor_copy(ot, psums[qc])
                            nc.sync.dma_start(
                                out=odram[mc * 128:(mc + 1) * 128,
                                          qc * 512:(qc + 1) * 512],
                                in_=ot,
                            )
                        yield

            qkv = gen_qkv()
            for qt in range(QT):
                # score group qt: rel rows for q-tile qt
                sps = [
                    pspool.tile([128, 512], F32, tag=f"s{kc}", name=f"s{kc}")
                    for kc in range(NQ)
                ]
                for kc in range(NQ):
                    for d in range(DC):
                        nc.tensor.matmul(
                            sps[kc],
                            qpi[d][:, qt * 128:(qt + 1) * 128],
                            kpi[d][:, kc * 512:(kc + 1) * 512],
                            start=(d == 0), stop=(d == DC - 1),
                        )
                relcols = rcpool.tile([128, NQ], F32, tag="relcols",
                                      name="relcols")
                for kc in range(NQ):
                    scratch = scpool.tile([128, 512], F16, tag="scratch",
                                          name="scratch")
                    nc.scalar.activation(
                        scratch, sps[kc], AF.Relu,
                        accum_out=relcols[:, kc:kc + 1],
                    )
                nc.vector.tensor_reduce(
                    relmat[:, qt:qt + 1], relcols, axis=mybir.AxisListType.X,
                    op=OP.add,
                )
                next(qkv, None)
            for _ in qkv:
                pass

            nc.sync.dma_start(
                out=rel.rearrange("(t p) -> p t", p=128), in_=relmat
            )
    nc.compile()
    return nc


def build_lb(S=SEQ, H=HIDDEN, NHC=NUM_HEADS // N_CORES, HD=HEAD_DIM,
             window=LOCAL_WINDOW):
    """Per-core (attention heads): partial (S, H) f16 = softmax-attn @ Wo rows."""
    nc = bacc.Bacc("TRN2", target_bir_lowering=False, debug=False)
    KC, NQ, QT, OCC = S // 128, S // 512, S // 128, H // 512
    qTh = nc.dram_tensor("qTh", [NHC * HD, S], F16, kind="ExternalInput")
    kTh = nc.dram_tensor("kTh", [NHC * HD, S], F16, kind="ExternalInput")
    vh = nc.dram_tensor("vh", [S, NHC * HD], F16, kind="ExternalInput")
    vslh = nc.dram_tensor("vslh", [S, NHC * HD], F16, kind="ExternalInput")
    woh = nc.dram_tensor("woh", [NHC * HD, H], F16, kind="ExternalInput")
    hivec = nc.dram_tensor("hivec", [S], F16, kind="ExternalInput")
    selv = nc.dram_tensor("selv", [S], F16, kind="ExternalInput")
    part = nc.dram_tensor("part", [S, H], F16, kind="ExternalOutput")

    scale = 1.0 / math.sqrt(HD)
    AF = mybir.ActivationFunctionType
    OP = mybir.AluOpType

    with TileContext(nc) as tc:
        with (
            tc.tile_pool(name="const", bufs=1) as cpool,
            tc.tile_pool(name="qk", bufs=1) as qkpool,
            tc.tile_pool(name="vv", bufs=1) as vpool,
            tc.tile_pool(name="et", bufs=2) as etpool,
            tc.tile_pool(name="aon", bufs=1) as aopool,
            tc.tile_pool(name="dr", bufs=2) as drpool,
            tc.tile_pool(name="ev", bufs=2) as evpool,
            tc.tile_pool(name="ps", bufs=1, space="PSUM") as pspool,
        ):
            # head-0 k/q first (the first score matmuls' operands), then v,
            # THEN the hvec/svec scatter loads -- those are 2048 tiny
            # descriptors each, and their generation ahead of q/k delays PE
            # start; they aren't read until the first masks (~2us later).
            qsb, ksb, vhf, vsl = [], [], [], []
            for h in range(NHC):
                k = qkpool.tile([128, S], F16, name=f"ksb{h}")
                nc.sync.dma_start(out=k, in_=kTh[h * HD:(h + 1) * HD, :])
                ksb.append(k)
                q = qkpool.tile([128, S], F16, name=f"qsb{h}")
                nc.sync.dma_start(out=q, in_=qTh[h * HD:(h + 1) * HD, :])
                qsb.append(q)
                vt = vpool.tile([128, KC * HD], F16, name=f"vhf{h}")
                nc.sync.dma_start(
                    out=vt.rearrange("p (t d) -> p t d", t=KC),
                    in_=vh[:, h * HD:(h + 1) * HD].rearrange(
                        "(t p) d -> p t d", p=128),
                )
                vhf.append(vt)
                vs = vpool.tile([128, KC * HD], F16, name=f"vsl{h}")
                nc.sync.dma_start(
                    out=vs.rearrange("p (t d) -> p t d", t=KC),
                    in_=vslh[:, h * HD:(h + 1) * HD].rearrange(
                        "(t p) d -> p t d", p=128),
                )
                vsl.append(vs)
                if h == 0:
                    hvec = cpool.tile([128, KC], F16, name="hvec")
                    nc.sync.dma_start(
                        out=hvec, in_=hivec.rearrange("(t p) -> p t", p=128))
                    svec = cpool.tile([128, KC], F16, name="svec")
                    nc.sync.dma_start(
                        out=svec, in_=selv.rearrange("(t p) -> p t", p=128))

            iota = cpool.tile([128, S], F16, name="iota")
            nc.gpsimd.iota(
                iota, pattern=[[1, S]], base=0, channel_multiplier=0,
                allow_small_or_imprecise_dtypes=True,
            )
            ones = cpool.tile([128, 1], F16, name="ones")
            nc.vector.memset(ones, 1.0)

            wsb = []
            for h in range(NHC):
                w = qkpool.tile([128, H], F16, name=f"wsb{h}")
                nc.sync.dma_start(out=w, in_=woh[h * HD:(h + 1) * HD, :])
                wsb.append(w)

            aon = [aopool.tile([128, S], F16, name=f"aon{h}")
                   for h in range(NHC)]

            def make_norm(h, avp, den128):
                def emit_norm(qc):
                    # den -> reciprocal -> partition-broadcast -> normalize;
                    # DVE/gpsimd only, so PE never waits on this chain except
                    # through the av/den bank reuse semaphores.
                    q0 = qc * 512
                    dq = drpool.tile([1, 512], F32, tag=f"dq{qc}",
                                     name=f"dq{qc}")
                    nc.vector.tensor_copy(dq, den128[32 * qc:32 * qc + 1, :])
                    rq = drpool.tile([1, 512], F32, tag=f"rq{qc}",
                                     name=f"rq{qc}")
                    rs = drpool.tile([1, 512], F32, tag=f"rs{qc}",
                                     name=f"rs{qc}")
                    nc.vector.reciprocal_approx_accurate(rq, dq, rs)
                    rbs = drpool.tile([128, 512], F32, tag="rbs", name="rbs")
                    nc.gpsimd.partition_broadcast(rbs, rq)
                    nc.vector.scalar_tensor_tensor(
                        aon[h][:, q0:q0 + 512], rbs, 1.0, avp[qc],
                        op0=OP.mult, op1=OP.mult,
                    )
                return emit_norm

            for h in range(NHC):
                avp = [
                    pspool.tile([128, 512], F32, tag=f"av{qc}", bufs=1,
                                name=f"av{qc}")
                    for qc in range(NQ)
                ]
                den128 = pspool.tile([128, 512], F32, tag="den", bufs=1,
                                     name="den128")
                emit_norm = make_norm(h, avp, den128)

                def emit_av_den(kc, qcs, far, ets):
                    for qc in qcs:
                        lhs_av = vsl[h] if far[qc] else vhf[h]
                        nc.tensor.matmul(
                            avp[qc], lhs_av[:, kc * 128:(kc + 1) * 128],
                            ets[qc], start=(kc == 0),
                            stop=(kc == (qc * 512 + 511) // 128),
                        )
                    for qc in qcs:
                        lhs_den = svec[:, kc:kc + 1] if far[qc] else ones
                        nc.tensor.matmul(
                            den128[32 * qc:32 * qc + 1, :], lhs_den, ets[qc],
                            start=(kc == 0),
                            stop=(kc == (qc * 512 + 511) // 128),
                            tile_position=(0, 32 * qc),
                        )
                    # a q-chunk whose last key tile just finished can be
                    # normalized now, overlapping the remaining kc loop
                    for qc in qcs:
                        if kc == (qc * 512 + 511) // 128:
                            emit_norm(qc)

                pend = None
                for kc in range(KC):
                    k0 = kc * 128
                    qcs = [qc for qc in range(NQ) if qc * 512 + 511 >= k0]
                    far = {qc: qc * 512 > k0 + 127 + window for qc in qcs}
                    ets = {}
                    for qc in qcs:
                        q0 = qc * 512
                        diag = not far[qc] and q0 < k0 + 128
                        # on the diagonal tile, cols [q0, k0) are entirely
                        # causally masked: skip their score matmul and exp
                        # (PE and ACT are the tight engines here);
                        # affine_select writes 0 there anyway.
                        pre = k0 - q0 if diag else 0
                        sps = pspool.tile([128, 512], F32, tag="sc", bufs=3,
                                          name="sps")
                        nc.tensor.matmul(
                            sps[:, pre:], ksb[h][:, k0:k0 + 128],
                            qsb[h][:, q0 + pre:q0 + 512],
                            start=True, stop=True,
                        )
                        et = etpool.tile([128, 512], F16, tag=f"et{qc}",
                                         name=f"et{qc}")
                        ets[qc] = et
                        nc.scalar.activation(
                            et[:, pre:], sps[:, pre:], AF.Exp, scale=scale)
                        if far[qc]:
                            continue  # sel-mask folded into vsl/svec operands
                        if diag:
                            # causal: zero where q < k (fills the skipped
                            # prefix too)
                            nc.gpsimd.affine_select(
                                out=et, in_=et, compare_op=OP.is_ge, fill=0.0,
                                base=q0 - k0, channel_multiplier=-1,
                                pattern=[[1, 512]],
                            )
                        if q0 + 511 > k0 + window:
                            nc.vector.scalar_tensor_tensor(
                                et, iota[:, q0:q0 + 512], hvec[:, kc:kc + 1],
                                et, op0=OP.is_le, op1=OP.mult,
                            )
                    if pend is not None:
                        emit_av_den(*pend)
                    pend = (kc, qcs, far, ets)
                emit_av_den(*pend)

            # output projection: partial = sum_h aon_h @ Wo rows.
            # oc=3 uses the den bank so no wops waits on the (late) av3
            # normalize read; qt order is free since all norms are done.
            wop_tags = ["av0", "av1", "av2", "den"]
            nev = 0
            for qt in range(QT):
                wops = [
                    pspool.tile([128, 512], F32, tag=wop_tags[oc], bufs=1,
                                name=f"wops{oc}")
                    for oc in range(OCC)
                ]
                for h in range(NHC):
                    for oc in range(OCC):
                        nc.tensor.matmul(
                            wops[oc], aon[h][:, qt * 128:(qt + 1) * 128],
                            wsb[h][:, oc * 512:(oc + 1) * 512],
                            start=(h == 0), stop=(h == NHC - 1),
                        )
                for oc in range(OCC):
                    ot = evpool.tile([128, 512], F16, tag=f"ot{oc}",
                                     name=f"ot{oc}")
                    nev += 1
                    if nev % 2 == 0:
                        nc.scalar.copy(ot, wops[oc])
                    else:
                        nc.vector.tensor_copy(ot, wops[oc])
                    nc.sync.dma_start(
                        out=part[qt * 128:(qt + 1) * 128,
                                 oc * 512:(oc + 1) * 512],
                        in_=ot,
                    )
    nc.compile()
    return nc


_CACHE = {}


def _get(name, builder, *args):
    key = (name,) + args
    if key not in _CACHE:
        _CACHE[key] = builder(*args)
    return _CACHE[key]


def _run(nc, in_maps):
    res = run_bass_kernel_spmd(
        nc, in_maps, core_ids=list(range(N_CORES)), trace=_TRACE["on"]
    )
    if _TRACE["on"] and res.exec_time_ns is not None:
        _TRACE["exec_ns"].append(res.exec_time_ns)
    return res.results


def kernel(hidden_states, Wq, Wk, Wv, Wo, Wq_ind, Wk_ind, head_weights,
           temperature_param):
    hidden_states = np.asarray(hidden_states, dtype=FP32)
    Wq, Wk, Wv, Wo = (np.asarray(a, dtype=FP32) for a in (Wq, Wk, Wv, Wo))
    Wq_ind = np.asarray(Wq_ind, dtype=FP32)
    Wk_ind = np.asarray(Wk_ind, dtype=FP32)
    head_weights = np.asarray(head_weights, dtype=FP32)

    B, S, H = hidden_states.shape
    assert B == 1 and H == HIDDEN and S == SEQ
    CS = H // N_CORES

    # fused indexer weights: qp = q_lin@Wq_ind = hidden@(Wq@Wq_ind)
    Wqi_f = Wq @ Wq_ind
    Wki_f = Wk @ Wk_ind

    hidT = np.ascontiguousarray(hidden_states[0].T).astype(np.float16)

    # ---- launch A: projections + indexer rel ----
    nca = _get("la", build_la, S, H, CS)
    ina = [
        {
            "hidT": hidT,
            "wq": np.ascontiguousarray(Wq[:, c * CS:(c + 1) * CS]).astype(np.float16),
            "wk": np.ascontiguousarray(Wk[:, c * CS:(c + 1) * CS]).astype(np.float16),
            "wv": np.ascontiguousarray(Wv[:, c * CS:(c + 1) * CS]).astype(np.float16),
            "wqi": np.ascontiguousarray(Wqi_f[:, c * CS:(c + 1) * CS]).astype(np.float16),
            "wki": np.ascontiguousarray(Wki_f[:, c * CS:(c + 1) * CS]).astype(np.float16),
        }
        for c in range(N_CORES)
    ]
    ra = _run(nca, ina)

    rel = np.zeros(S, dtype=np.float64)
    for c in range(N_CORES):
        rel += float(head_weights[c]) * np.asarray(ra[c]["rel"], dtype=np.float64)
    # exp(-temp) scaling is monotone; irrelevant for top-k selection.

    k_sel = min(MAX_SELECTED, S)
    top_idx = np.argpartition(-rel, k_sel - 1)[:k_sel]
    selected = np.zeros(S, dtype=bool)
    selected[top_idx] = True

    # ---- launch B: masked attention + output projection ----
    BIG = float(2 * S + 1024)
    hi = np.where(selected, BIG, np.arange(S, dtype=np.float64) + LOCAL_WINDOW)
    hi = hi.astype(np.float16)
    selv = selected.astype(np.float16)
    NHC = NUM_HEADS // N_CORES
    RW = NHC * HEAD_DIM

    ncb = _get("lb", build_lb, S, H, NHC, HEAD_DIM, LOCAL_WINDOW)
    inb = []
    for c in range(N_CORES):
        vhc = np.ascontiguousarray(
            np.asarray(ra[c]["vT"], dtype=np.float16).T)  # (S, 256) key-major
        inb.append({
            "qTh": np.asarray(ra[c]["qT"]),
            "kTh": np.asarray(ra[c]["kT"]),
            "vh": vhc,
            "vslh": np.ascontiguousarray(vhc * selv[:, None]),
            "woh": np.ascontiguousarray(Wo[c * RW:(c + 1) * RW]).astype(np.float16),
            "hivec": hi,
            "selv": selv,
        })
    rb = _run(ncb, inb)
    out = np.zeros((S, H), dtype=np.float32)
    for c in range(N_CORES):
        out += np.asarray(rb[c]["part"], dtype=np.float32)
    return out.reshape(B, S, H).astype(np.float32)
